# revision 19
# baseline (speedup 1.0000x reference)
"""Multi-head attention (B=4, S=2048, D=1024, H=16) on 8 trn2 NeuronCores.

Sharding: 8 cores = 4 batches x 2 head-groups. Core c handles batch c//2 and
heads [8g, 8g+8) where g = c%2 (tensor-parallel: Wq/Wk/Wv column-sliced,
Wo row-sliced). Each core returns a partial output [S, D]; the host sums the
two head-group partials per batch.

Host-side prep: keys/values are COMPACTED per batch (mask==1 keys contribute
exp(score-1e9) == 0 exactly, so they are dropped and the k/v streams padded to
SK = ceil(alive/128)*128 with masked pad rows). q/k/v are transposed to
[D, S]-major on the host and everything is cast to bf16, so the device does
plain sprayed DMA loads and runs all matmuls at full bf16 PE rate.

Per-core dataflow (everything stays transposed until the output projection):
  X.T loaded directly -> Q.T/K.T = W.T @ X.T (bf16), V natural (bf16, ones
  column appended) -> scores.T = K @ Q.T (row-tiled concurrent head pairs) ->
  exp+mask+scale in one ACT op -> ctxU.T = V'.T @ expS.T (last row = softmax
  denominator) -> normalize (DMA denom to partition 0, DVE
  reciprocal_approx_fast, gpsimd broadcast) -> out = ctx.T.T @ Wo + bo.

Schedule: V and m=0 slices of K.T/Q.T are projected up front; the remaining
projection slices, then next-superchunk Q.T, then out-projection chunks are
pumped into the ACT-bound attention loop as PE filler. Attention iterates
q-chunks outermost so out-projection lags attention by one q-chunk.
"""

import sys

if "/opt/trn_rl_repo" not in sys.path:
    sys.path.append("/opt/trn_rl_repo")

import numpy as np
import ml_dtypes

import concourse.bass as bass
import concourse.bacc as bacc
import concourse.tile as tile
from concourse import mybir
from concourse.bass import ts

F32 = mybir.dt.float32
F32R = mybir.dt.float32r
BF16 = mybir.dt.bfloat16
I32 = mybir.dt.int32
EXP = mybir.ActivationFunctionType.Exp

P = 128


def build_nc(S=2048, D=1024, DL=512, HD=64, SKT=9):
    """Per-core Bass program. DL = local output dim; SKT = key token tiles."""
    SK = SKT * P  # compacted+padded key tokens
    KD = D // P  # contraction tiles over D
    MT = DL // P  # local d-col tiles
    HL = DL // HD  # local heads
    HPT = P // HD  # heads per 128-partition tile (2)
    NCH = min(512, S)  # q-projection token chunk
    QS = min(1024, S)  # attention q superchunk
    QH = min(512, QS)  # one-bank column chunk
    NH = QS // QH
    NQ = S // QS
    OC = min(512, D)  # out-proj col chunk
    kchunks = []
    off = 0
    while off < SK:
        w = min(512, SK - off)
        kchunks.append((off, w))
        off += w
    qchunks = [(i * NCH, NCH) for i in range(S // NCH)]
    scale = float(1.0 / (np.sqrt(np.float32(HD)) + 1e-8))

    nc = bacc.Bacc("TRN2", target_bir_lowering=False, debug=False)

    xqt = nc.dram_tensor("xqt", [D, S], BF16, kind="ExternalInput")
    xkt = nc.dram_tensor("xkt", [D, SK], BF16, kind="ExternalInput")
    xvt = nc.dram_tensor("xvt", [D, SK], BF16, kind="ExternalInput")
    msk = nc.dram_tensor("msk", [P, SKT], I32, kind="ExternalInput")
    wq = nc.dram_tensor("wq", [D, DL], BF16, kind="ExternalInput")
    wk = nc.dram_tensor("wk", [D, DL], BF16, kind="ExternalInput")
    wv = nc.dram_tensor("wv", [D, DL], BF16, kind="ExternalInput")
    wo = nc.dram_tensor("wo", [DL, D], BF16, kind="ExternalInput")
    bq = nc.dram_tensor("bq", [P, MT], F32, kind="ExternalInput")
    bk = nc.dram_tensor("bk", [P, MT], F32, kind="ExternalInput")
    bv = nc.dram_tensor("bv", [1, DL], F32, kind="ExternalInput")
    bo = nc.dram_tensor("bo", [1, D], F32, kind="ExternalInput")
    out = nc.dram_tensor("out", [S, D], F32, kind="ExternalOutput")

    with tile.TileContext(nc) as tc, nc.allow_low_precision("bf16 compute by design"):
        with (
            tc.tile_pool(name="pers", bufs=1) as pers,
            tc.tile_pool(name="wpool", bufs=1) as wpool,
            tc.tile_pool(name="exp", bufs=4) as ex_pool,
            tc.tile_pool(name="osb", bufs=2) as osb_pool,
            tc.tile_pool(name="small", bufs=2) as small,
        ):
            # ---- constants ----
            ones0 = pers.tile([1, P], F32, tag="ones0")
            nc.gpsimd.memset(ones0[:], 1.0)
            ones = pers.tile([1, P], F32R, tag="ones")
            nc.vector.tensor_copy(out=ones[:], in_=ones0[:])

            mi = pers.tile([P, SKT], I32, tag="mi")
            nc.sync.dma_start(mi[:], msk[:, :])
            mf = pers.tile([P, SKT], F32, tag="mf")
            nc.vector.tensor_copy(out=mf[:], in_=mi[:])
            mb = pers.tile([P, SKT], F32, tag="mb")
            nc.vector.tensor_scalar_mul(mb[:], mf[:], -1.0e9)
            wrm = pers.tile([1, P], F32, tag="wrm")
            nc.scalar.activation(wrm[:], ones0[:], EXP)

            bqs = pers.tile([P, MT], F32, tag="bqs")
            nc.sync.dma_start(bqs[:], bq[:, :])
            bks = pers.tile([P, MT], F32, tag="bks")
            nc.sync.dma_start(bks[:], bk[:, :])
            bvstg = small.tile([1, DL], F32, tag="bvstg", name="bvstg", bufs=1)
            nc.sync.dma_start(bvstg[0:1, :], bv[:, :])
            bvs = pers.tile([1, DL], F32R, tag="bvs")
            nc.vector.tensor_copy(out=bvs[:], in_=bvstg[0:1, :])
            bostg = small.tile([1, D], F32, tag="bostg", name="bostg", bufs=1)
            nc.sync.dma_start(bostg[:], bo[:, :])
            bos = pers.tile([1, D], F32R, tag="bos")
            nc.vector.tensor_copy(out=bos[:], in_=bostg[:])
            bvb = pers.tile([P, DL], F32, tag="bvb")
            bob = pers.tile([P, D], F32, tag="bob")

            # ---- X.T loads (host pre-transposed, bf16), ordered so the
            # phase-1 gates (XKT+wk, then wv+XVT, then XQT halves) land first;
            # wos (out-proj weights) is only needed mid-attention
            wks = wpool.tile([P, KD, DL], BF16, tag="w", name="w")
            wvs = wpool.tile([P, KD, DL], BF16, tag="w2", name="w2")
            wqs = wpool.tile([P, KD, DL], BF16, tag="w3", name="w3")
            XKT = pers.tile([P, KD, SK], BF16, tag="xkt")
            nc.sync.dma_start(XKT[:], xkt.rearrange("(k p) s -> p k s", p=P))
            nc.sync.dma_start(wks[:], wk.rearrange("(k p) n -> p k n", p=P))
            XVT = pers.tile([P, KD, SK], BF16, tag="xvt")
            nc.sync.dma_start(wvs[:], wv.rearrange("(k p) n -> p k n", p=P))
            nc.sync.dma_start(XVT[:], xvt.rearrange("(k p) s -> p k s", p=P))
            XQT = pers.tile([P, KD, S], BF16, tag="xqt")
            nc.sync.dma_start(XQT[:, :, 0 : S // 2], xqt[:, 0 : S // 2].rearrange("(k p) s -> p k s", p=P))
            nc.sync.dma_start(wqs[:], wq.rearrange("(k p) n -> p k n", p=P))
            nc.sync.dma_start(XQT[:, :, S // 2 : S], xqt[:, S // 2 : S].rearrange("(k p) s -> p k s", p=P))
            wos = pers.tile([P, MT, D], BF16, tag="wos")
            nc.sync.dma_start(wos[:], wo.rearrange("(m p) n -> p m n", p=P))

            # persistent activation stores
            KT = [pers.tile([P, SK], BF16, tag=f"kt{m}", name=f"kt{m}") for m in range(MT)]
            QT = [pers.tile([P, S], BF16, tag=f"qt{m}", name=f"qt{m}") for m in range(MT)]
            CT = [pers.tile([P, S], BF16, tag=f"ct{m}", name=f"ct{m}") for m in range(MT)]
            VP = [pers.tile([P, HL * (HD + 1)], BF16, tag=f"vp{t}", name=f"vp{t}") for t in range(SKT)]
            for t in range(SKT):
                nc.gpsimd.memset(VP[t][:], 1.0)

            def proj_unit(XT, wsb, bias_sb, dst_tiles, c0, cw, m, acc_pool):
                """dst[m][:, c0:c0+cw] = ((x @ w).T + b)[m-rows, chunk]."""
                acc = acc_pool.tile([P, QH], F32, tag="acc", name="pacc")
                for kk in range(KD):
                    nc.tensor.matmul(
                        acc[:, 0:cw],
                        lhsT=wsb[:, kk, ts(m, P)],
                        rhs=XT[:, kk, c0 : c0 + cw],
                        start=(kk == 0),
                        stop=(kk == KD - 1),
                    )
                nc.vector.tensor_scalar_add(
                    dst_tiles[m][:, c0 : c0 + cw], acc[:, 0:cw], bias_sb[:, m : m + 1]
                )

            def vproj(wsb, acc_pool):
                """VP[t][:, h*(HD+1)+1:+HD] = (xv @ wv + bv)[t-tile, h-slice]."""
                for t in range(SKT):
                    acc = acc_pool.tile([P, DL], F32, tag="acc", name="vacc")
                    for kk in range(KD):
                        nc.tensor.matmul(
                            acc[:],
                            lhsT=XVT[:, kk, ts(t, P)],
                            rhs=wsb[:, kk, :],
                            start=(kk == 0),
                            stop=(kk == KD - 1),
                        )
                    for h in range(HL):
                        nc.vector.tensor_add(
                            VP[t][:, h * (HD + 1) : h * (HD + 1) + HD],
                            acc[:, ts(h, HD)],
                            bvb[:, ts(h, HD)],
                        )

            def attn_chunk(qq, q5, sc_pool, cx_pool, pending, filler=None,
                           pump_every=8, pump_offset=0, defer_tails=True):
                """Attention for q-columns [qq*QS + q5*QH, +QH), all head pairs."""
                # flush tails carried over from the previous chunk so filler
                # out-projections see completed CT columns
                for fn in pending:
                    fn()
                del pending[:]
                it = 0
                col0 = qq * QS + q5 * QH
                for hp in range(HL // HPT):
                    cxs = [
                        cx_pool.tile([HD + 1, QH], F32, tag="cx", name="cx")
                        for _ in range(HPT)
                    ]
                    for kt in range(SKT):
                        sc = sc_pool.tile([P, HPT * QH], F32, tag="sc")
                        for u in range(HPT):
                            mo = u * HD
                            nc.tensor.matmul(
                                sc[:, ts(u, QH)],
                                lhsT=KT[hp][mo : mo + HD, ts(kt, P)],
                                rhs=QT[hp][mo : mo + HD, col0 : col0 + QH],
                                start=True,
                                stop=True,
                            )
                        ex = ex_pool.tile([P, HPT * QH], BF16, tag="ex")
                        nc.scalar.activation(
                            ex[:], sc[:], EXP, bias=mb[:, kt : kt + 1], scale=scale
                        )
                        for u in range(HPT):
                            h = hp * HPT + u
                            nc.tensor.matmul(
                                cxs[u][:],
                                lhsT=VP[kt][:, h * (HD + 1) : (h + 1) * (HD + 1)],
                                rhs=ex[:, ts(u, QH)],
                                start=(kt == 0),
                                stop=(kt == SKT - 1),
                            )
                        it += 1
                        if (
                            filler is not None
                            and it > pump_offset
                            and (it - pump_offset) % pump_every == 0
                        ):
                            next(filler, None)
                    prev_tails = pending[:]
                    del pending[:]
                    for u in range(HPT):
                        mo = u * HD
                        # cheap DVE copy frees the ctx PSUM slot; the rest of
                        # the normalize is deferred one unit
                        stg = small.tile(
                            [HD + 1, QH], F32, tag="stg", name="stg", bufs=4
                        )
                        nc.vector.tensor_copy(out=stg[:], in_=cxs[u][:])

                        def tail(hp=hp, mo=mo, col0=col0, stg=stg):
                            # denominator row lives on partition HD; gpsimd
                            # broadcast and custom-DVE ops only read base
                            # partition 0, so DMA it there first
                            den = small.tile([1, QH], F32, tag="den", bufs=2)
                            nc.sync.dma_start(den[0:1, :], stg[HD : HD + 1, :])
                            rec1 = small.tile([1, QH], F32, tag="rec1", bufs=2)
                            nc.vector.reciprocal_approx_fast(rec1[:], den[:])
                            rec = small.tile([HD, QH], F32, tag="rec", bufs=2)
                            nc.gpsimd.partition_broadcast(rec[:], rec1[0:1, :])
                            if mo == 0:
                                nc.vector.tensor_mul(
                                    CT[hp][0:HD, col0 : col0 + QH], stg[0:HD, :], rec[:]
                                )
                            else:
                                tmp = small.tile([HD, QH], BF16, tag="tmp")
                                nc.vector.tensor_mul(tmp[:], stg[0:HD, :], rec[:])
                                nc.sync.dma_start(
                                    CT[hp][mo : mo + HD, col0 : col0 + QH], tmp[:]
                                )

                        if defer_tails:
                            pending.append(tail)
                        else:
                            tail()
                    for fn in prev_tails:
                        fn()
                # drain remaining filler units
                if filler is not None:
                    for _ in filler:
                        pass

            def outproj_units(qq, q5, acc_pool):
                """Out-projection for the q-token window covered by (qq, q5)."""
                t0 = (qq * QS + q5 * QH) // P
                for t in range(t0, t0 + QH // P):
                    for c in range(D // OC):
                        po = acc_pool.tile([P, OC], F32, tag="acc", name="po")
                        for dd in range(MT):
                            nc.tensor.matmul(
                                po[:],
                                lhsT=CT[dd][:, ts(t, P)],
                                rhs=wos[:, dd, ts(c, OC)],
                                start=(dd == 0),
                                stop=(dd == MT - 1),
                            )
                        osb = osb_pool.tile([P, OC], F32, tag="osb")
                        nc.vector.tensor_add(osb[:], po[:], bob[:, ts(c, OC)])
                        nc.sync.dma_start(out[ts(t, P), ts(c, OC)], osb[:])
                        yield

            # ---- phase 1: V', K.T m=0, Q.T sc0 m=0 (attention prerequisites)
            with tc.tile_pool(name="ps1", bufs=4, space="PSUM") as ps1:
                for c in range(D // OC):
                    bp = ps1.tile([P, OC], F32, tag="acc", name="bp")
                    nc.tensor.matmul(
                        bp[:], lhsT=ones[0:1, 0:P], rhs=bos[0:1, ts(c, OC)],
                        start=True, stop=True,
                    )
                    nc.vector.tensor_copy(out=bob[:, ts(c, OC)], in_=bp[:])
                bp = ps1.tile([P, DL], F32, tag="acc", name="bp2")
                nc.tensor.matmul(
                    bp[:], lhsT=ones[0:1, 0:P], rhs=bvs[0:1, :], start=True, stop=True
                )
                nc.vector.tensor_copy(out=bvb[:], in_=bp[:])
                for c0, cw in kchunks:
                    proj_unit(XKT, wks, bks, KT, c0, cw, 0, ps1)
                vproj(wvs, ps1)

            # ---- phase 2: attention with projection/out-proj filler ----
            with (
                tc.tile_pool(name="ps2sc", bufs=2, space="PSUM") as ps2sc,
                tc.tile_pool(name="ps2cx", bufs=3, space="PSUM") as ps2cx,
                tc.tile_pool(name="ps2q", bufs=1, space="PSUM") as ps2q,
            ):
                CPQ = QS // NCH  # q-proj chunks per superchunk
                n_its = (HL // HPT) * SKT  # attention kt-iterations per chunk
                for c0, cw in qchunks[:CPQ]:
                    proj_unit(XQT, wqs, bqs, QT, c0, cw, 0, ps2q)

                def units(specs):
                    for XT, wsb, bsb, dst, c0, cw, m in specs:
                        proj_unit(XT, wsb, bsb, dst, c0, cw, m, ps2q)
                        yield

                def _mix(a, b):
                    from itertools import chain, zip_longest

                    yield from chain.from_iterable(zip_longest(a, b))

                # chunk sequence: (0,0), (0,1), ..., (NQ-1, NH-1).
                # filler plans keep QT[m]/KT[m] producers strictly ahead of the
                # attention head pair that consumes them (in-order PE queue).
                pending = []
                chunks2 = [(qq, q5) for qq in range(NQ) for q5 in range(NH)]
                for ci, (qq, q5) in enumerate(chunks2):
                    if ci == 0:
                        # remaining K.T slices + this chunk's Q.T column slices
                        specs = []
                        for m in range(1, MT):
                            specs.append((XQT, wqs, bqs, QT, qchunks[0][0], qchunks[0][1], m))
                            for c0, cw in kchunks:
                                specs.append((XKT, wks, bks, KT, c0, cw, m))
                        filler = units(specs)
                        pe, po = 2, 0
                    else:
                        specs = []
                        if ci == 1 and NH * NCH >= QS:
                            # q5=1 column slices for m>=1 (needed by this
                            # chunk's later head pairs)
                            for m in range(1, MT):
                                specs.append(
                                    (XQT, wqs, bqs, QT, qchunks[1][0], qchunks[1][1], m)
                                )
                        if qq + 1 < NQ and q5 == NH - 1:
                            # next superchunk's first-needed Q.T columns, all m
                            for m in range(MT):
                                specs.append(
                                    (XQT, wqs, bqs, QT, *qchunks[(qq + 1) * CPQ], m)
                                )
                        if q5 == 0 and qq >= 1:
                            # this superchunk's later Q.T columns were deferred
                            # to here; also pre-project the next superchunk's
                            # later columns if any
                            for m in range(MT):
                                for ck in qchunks[qq * CPQ + 1 : (qq + 1) * CPQ]:
                                    specs.append((XQT, wqs, bqs, QT, *ck, m))
                        pq, p5 = chunks2[ci - 1]
                        ogen = outproj_units(pq, p5, ps2q)
                        filler = _mix(units(specs), ogen) if specs else ogen
                        nun = len(specs) + QH // P * (D // OC)
                        pe, po = max(1, (n_its - 2) // nun), 2
                    attn_chunk(
                        qq, q5, ps2sc, ps2cx, pending, filler,
                        pump_every=pe, pump_offset=po,
                        defer_tails=(ci < len(chunks2) - 1),
                    )
                for fn in pending:
                    fn()
                for _ in outproj_units(NQ - 1, NH - 1, ps2q):
                    pass

    nc.compile()
    return nc


_NC_CACHE = {}


def _get_nc(S, D, DL, HD, SKT):
    key = (S, D, DL, HD, SKT)
    if key not in _NC_CACHE:
        _NC_CACHE[key] = build_nc(S, D, DL, HD, SKT)
    return _NC_CACHE[key]


def _shard_inputs(q, k, v, mask, Wq, bq, Wk, bk, Wv, bv, Wo, bo):
    q, k, v = np.asarray(q), np.asarray(k), np.asarray(v)
    mask = np.asarray(mask)
    Wq, Wk, Wv, Wo = np.asarray(Wq), np.asarray(Wk), np.asarray(Wv), np.asarray(Wo)
    bq, bk, bv, bo = np.asarray(bq), np.asarray(bk), np.asarray(bv), np.asarray(bo)

    B, S, D = q.shape  # 4, 2048, 1024
    G = 2  # head-groups (tensor-parallel factor); B*G = 8 cores
    DL = D // G
    MT = DL // P

    bf16 = ml_dtypes.bfloat16
    f32 = np.float32

    # compact keys/values: masked keys contribute exp(score-1e9) == 0 exactly
    m2 = mask[:, 0, 0, :]  # [B, S], 1 = masked
    idxs = [np.nonzero(m2[b] == 0)[0] for b in range(B)]
    SKT = max(1, -(-max(len(ix) for ix in idxs) // P))
    SK = SKT * P

    qt = [np.ascontiguousarray(q[b].T.astype(bf16)) for b in range(B)]
    kt, vt, mk = [], [], []
    for b in range(B):
        ix = idxs[b]
        n = len(ix)
        kc = np.zeros((D, SK), dtype=bf16)
        kc[:, :n] = k[b][ix].T.astype(bf16)
        vc = np.zeros((D, SK), dtype=bf16)
        vc[:, :n] = v[b][ix].T.astype(bf16)
        kt.append(kc)
        vt.append(vc)
        mk.append(
            np.ascontiguousarray(
                (np.arange(SK) >= n).astype(np.int32).reshape(SKT, P).T
            )
        )

    in_maps = []
    for c in range(B * G):
        b, g = c // G, c % G
        sl = slice(g * DL, (g + 1) * DL)
        bo_core = bo if g == 0 else np.zeros_like(bo)
        in_maps.append(
            {
                "xqt": qt[b],
                "xkt": kt[b],
                "xvt": vt[b],
                "msk": mk[b],
                "wq": np.ascontiguousarray(Wq[:, sl]).astype(bf16),
                "wk": np.ascontiguousarray(Wk[:, sl]).astype(bf16),
                "wv": np.ascontiguousarray(Wv[:, sl]).astype(bf16),
                "wo": np.ascontiguousarray(Wo[sl, :]).astype(bf16),
                "bq": np.ascontiguousarray(bq[sl].reshape(MT, P).T, dtype=f32),
                "bk": np.ascontiguousarray(bk[sl].reshape(MT, P).T, dtype=f32),
                "bv": np.ascontiguousarray(bv[sl].reshape(1, DL), dtype=f32),
                "bo": np.ascontiguousarray(bo_core.reshape(1, D), dtype=f32),
            }
        )
    return in_maps, SKT


def kernel(q, k, v, mask, Wq, bq, Wk, bk, Wv, bv, Wo, bo):
    from concourse.bass_utils import run_bass_kernel_spmd

    q = np.asarray(q)
    B, S, D = q.shape  # 4, 2048, 1024
    G = 2
    in_maps, SKT = _shard_inputs(q, k, v, mask, Wq, bq, Wk, bk, Wv, bv, Wo, bo)
    nc = _get_nc(S, D, D // G, 64, SKT)

    res = run_bass_kernel_spmd(nc, in_maps, core_ids=list(range(B * G)))
    parts = [r["out"] for r in res.results]
    outf = np.stack([parts[b * G] + parts[b * G + 1] for b in range(B)], axis=0)
    return outf.astype(np.float32)


# revision 20
# speedup vs baseline: 1.0686x; 1.0686x over previous
"""Multi-head attention (B=4, S=2048, D=1024, H=16) on 8 trn2 NeuronCores.

Sharding: 8 cores = 4 batches x 2 head-groups. Core c handles batch c//2 and
heads [8g, 8g+8) where g = c%2 (tensor-parallel: Wq/Wk/Wv column-sliced,
Wo row-sliced). Each core returns a partial output [S, D]; the host sums the
two head-group partials per batch.

Host-side prep: keys/values are COMPACTED per batch (mask==1 keys contribute
exp(score-1e9) == 0 exactly, so they are dropped and the k/v streams padded to
SK = ceil(alive/128)*128 with masked pad rows). q/k/v are transposed to
[D, S]-major on the host and everything is cast to bf16, so the device does
plain sprayed DMA loads and runs all matmuls at full bf16 PE rate.

Per-core dataflow (everything stays transposed until the output projection):
  X.T loaded directly -> Q.T/K.T = W.T @ X.T (bf16), V natural (bf16, ones
  column appended) -> scores.T = K @ Q.T (row-tiled concurrent head pairs) ->
  exp+mask+scale in one ACT op -> ctxU.T = V'.T @ expS.T (last row = softmax
  denominator) -> normalize (DMA denom to partition 0, DVE
  reciprocal_approx_fast, gpsimd broadcast) -> out = ctx.T.T @ Wo + bo.

Schedule: V and m=0 slices of K.T/Q.T are projected up front; the remaining
projection slices, then next-superchunk Q.T, then out-projection chunks are
pumped into the ACT-bound attention loop as PE filler. Attention iterates
q-chunks outermost so out-projection lags attention by one q-chunk.
"""

import sys

if "/opt/trn_rl_repo" not in sys.path:
    sys.path.append("/opt/trn_rl_repo")

import numpy as np
import ml_dtypes

import concourse.bass as bass
import concourse.bacc as bacc
import concourse.tile as tile
from concourse import mybir
from concourse.bass import ts

F32 = mybir.dt.float32
F32R = mybir.dt.float32r
BF16 = mybir.dt.bfloat16
I32 = mybir.dt.int32
EXP = mybir.ActivationFunctionType.Exp

P = 128


def build_nc(S=2048, D=1024, DL=512, HD=64, SKT=9):
    """Per-core Bass program. DL = local output dim; SKT = key token tiles."""
    SK = SKT * P  # compacted+padded key tokens
    KD = D // P  # contraction tiles over D
    MT = DL // P  # local d-col tiles
    HL = DL // HD  # local heads
    HPT = P // HD  # heads per 128-partition tile (2)
    NCH = min(512, S)  # q-projection token chunk
    QS = min(1024, S)  # attention q superchunk
    QH = min(512, QS)  # one-bank column chunk
    NH = QS // QH
    NQ = S // QS
    OC = min(512, D)  # out-proj col chunk
    kchunks = []
    off = 0
    while off < SK:
        w = min(512, SK - off)
        kchunks.append((off, w))
        off += w
    qchunks = [(i * NCH, NCH) for i in range(S // NCH)]
    scale = float(1.0 / (np.sqrt(np.float32(HD)) + 1e-8))

    nc = bacc.Bacc("TRN2", target_bir_lowering=False, debug=False)

    xqt = nc.dram_tensor("xqt", [D, S], BF16, kind="ExternalInput")
    xkt = nc.dram_tensor("xkt", [D, SK], BF16, kind="ExternalInput")
    xvt = nc.dram_tensor("xvt", [D, SK], BF16, kind="ExternalInput")
    msk = nc.dram_tensor("msk", [P, SKT], I32, kind="ExternalInput")
    wq = nc.dram_tensor("wq", [D, DL], BF16, kind="ExternalInput")
    wk = nc.dram_tensor("wk", [D, DL], BF16, kind="ExternalInput")
    wv = nc.dram_tensor("wv", [D, DL], BF16, kind="ExternalInput")
    wo = nc.dram_tensor("wo", [DL, D], BF16, kind="ExternalInput")
    bq = nc.dram_tensor("bq", [P, MT], F32, kind="ExternalInput")
    bk = nc.dram_tensor("bk", [P, MT], F32, kind="ExternalInput")
    bv = nc.dram_tensor("bv", [1, DL], F32, kind="ExternalInput")
    bo = nc.dram_tensor("bo", [1, D], F32, kind="ExternalInput")
    out = nc.dram_tensor("out", [S, D], F32, kind="ExternalOutput")

    with tile.TileContext(nc) as tc, nc.allow_low_precision("bf16 compute by design"):
        with (
            tc.tile_pool(name="pers", bufs=1) as pers,
            tc.tile_pool(name="wpool", bufs=1) as wpool,
            tc.tile_pool(name="exp", bufs=4) as ex_pool,
            tc.tile_pool(name="osb", bufs=2) as osb_pool,
            tc.tile_pool(name="small", bufs=2) as small,
        ):
            # ---- constants ----
            ones0 = pers.tile([1, P], F32, tag="ones0")
            nc.gpsimd.memset(ones0[:], 1.0)
            ones = pers.tile([1, P], F32R, tag="ones")
            nc.vector.tensor_copy(out=ones[:], in_=ones0[:])

            mi = pers.tile([P, SKT], I32, tag="mi")
            nc.sync.dma_start(mi[:], msk[:, :])
            mf = pers.tile([P, SKT], F32, tag="mf")
            nc.vector.tensor_copy(out=mf[:], in_=mi[:])
            mb = pers.tile([P, SKT], F32, tag="mb")
            nc.vector.tensor_scalar_mul(mb[:], mf[:], -1.0e9)
            wrm = pers.tile([1, P], F32, tag="wrm")
            nc.scalar.activation(wrm[:], ones0[:], EXP)

            bqs = pers.tile([P, MT], F32, tag="bqs")
            nc.sync.dma_start(bqs[:], bq[:, :])
            bks = pers.tile([P, MT], F32, tag="bks")
            nc.sync.dma_start(bks[:], bk[:, :])
            bvstg = small.tile([1, DL], F32, tag="bvstg", name="bvstg", bufs=1)
            nc.sync.dma_start(bvstg[0:1, :], bv[:, :])
            bvs = pers.tile([1, DL], F32R, tag="bvs")
            nc.vector.tensor_copy(out=bvs[:], in_=bvstg[0:1, :])
            bostg = small.tile([1, D], F32, tag="bostg", name="bostg", bufs=1)
            nc.sync.dma_start(bostg[:], bo[:, :])
            bos = pers.tile([1, D], F32R, tag="bos")
            nc.vector.tensor_copy(out=bos[:], in_=bostg[:])
            bvb = pers.tile([P, DL], F32, tag="bvb")
            bob = pers.tile([P, D], F32, tag="bob")

            # ---- X.T loads (host pre-transposed, bf16), ordered so the
            # phase-1 gates (XKT+wk, then wv+XVT, then XQT halves) land first;
            # wos (out-proj weights) is only needed mid-attention
            wks = wpool.tile([P, KD, DL], BF16, tag="w", name="w")
            wvs = wpool.tile([P, KD, DL], BF16, tag="w2", name="w2")
            wqs = wpool.tile([P, KD, DL], BF16, tag="w3", name="w3")
            XKT = pers.tile([P, KD, SK], BF16, tag="xkt")
            nc.sync.dma_start(XKT[:], xkt.rearrange("(k p) s -> p k s", p=P))
            nc.sync.dma_start(wks[:], wk.rearrange("(k p) n -> p k n", p=P))
            XVT = pers.tile([P, KD, SK], BF16, tag="xvt")
            nc.sync.dma_start(wvs[:], wv.rearrange("(k p) n -> p k n", p=P))
            nc.sync.dma_start(XVT[:], xvt.rearrange("(k p) s -> p k s", p=P))
            XQT = pers.tile([P, KD, S], BF16, tag="xqt")
            nc.sync.dma_start(XQT[:, :, 0 : S // 2], xqt[:, 0 : S // 2].rearrange("(k p) s -> p k s", p=P))
            nc.sync.dma_start(wqs[:], wq.rearrange("(k p) n -> p k n", p=P))
            nc.sync.dma_start(XQT[:, :, S // 2 : S], xqt[:, S // 2 : S].rearrange("(k p) s -> p k s", p=P))
            wos = pers.tile([P, MT, D], BF16, tag="wos")
            nc.sync.dma_start(wos[:], wo.rearrange("(m p) n -> p m n", p=P))

            # persistent activation stores
            KT = [pers.tile([P, SK], BF16, tag=f"kt{m}", name=f"kt{m}") for m in range(MT)]
            QT = [pers.tile([P, S], BF16, tag=f"qt{m}", name=f"qt{m}") for m in range(MT)]
            CT = [pers.tile([P, S], BF16, tag=f"ct{m}", name=f"ct{m}") for m in range(MT)]
            VP = [pers.tile([P, HL * (HD + 1)], BF16, tag=f"vp{t}", name=f"vp{t}") for t in range(SKT)]
            for t in range(SKT):
                nc.gpsimd.memset(VP[t][:], 1.0)

            def proj_unit(XT, wsb, bias_sb, dst_tiles, c0, cw, m, acc_pool):
                """dst[m][:, c0:c0+cw] = ((x @ w).T + b)[m-rows, chunk]."""
                acc = acc_pool.tile([P, QH], F32, tag="acc", name="pacc")
                for kk in range(KD):
                    nc.tensor.matmul(
                        acc[:, 0:cw],
                        lhsT=wsb[:, kk, ts(m, P)],
                        rhs=XT[:, kk, c0 : c0 + cw],
                        start=(kk == 0),
                        stop=(kk == KD - 1),
                    )
                nc.vector.tensor_scalar_add(
                    dst_tiles[m][:, c0 : c0 + cw], acc[:, 0:cw], bias_sb[:, m : m + 1]
                )

            def vproj(wsb, acc_pool):
                """VP[t][:, h*(HD+1)+1:+HD] = (xv @ wv + bv)[t-tile, h-slice]."""
                for t in range(SKT):
                    acc = acc_pool.tile([P, DL], F32, tag="acc", name="vacc")
                    for kk in range(KD):
                        nc.tensor.matmul(
                            acc[:],
                            lhsT=XVT[:, kk, ts(t, P)],
                            rhs=wsb[:, kk, :],
                            start=(kk == 0),
                            stop=(kk == KD - 1),
                        )
                    for h in range(HL):
                        nc.vector.tensor_add(
                            VP[t][:, h * (HD + 1) : h * (HD + 1) + HD],
                            acc[:, ts(h, HD)],
                            bvb[:, ts(h, HD)],
                        )

            def attn_chunk(qq, q5, sc_pool, cx_pool, pending, filler=None,
                           pump_every=8, pump_offset=0, defer_tails=True):
                """Attention for q-columns [qq*QS + q5*QH, +QH), all head pairs."""
                # flush tails carried over from the previous chunk so filler
                # out-projections see completed CT columns
                for fn in pending:
                    fn()
                del pending[:]
                it = 0
                col0 = qq * QS + q5 * QH
                for hp in range(HL // HPT):
                    cxs = [
                        cx_pool.tile([HD + 1, QH], F32, tag="cx", name="cx")
                        for _ in range(HPT)
                    ]
                    for kt in range(SKT):
                        sc = sc_pool.tile([P, HPT * QH], F32, tag="sc")
                        for u in range(HPT):
                            mo = u * HD
                            nc.tensor.matmul(
                                sc[:, ts(u, QH)],
                                lhsT=KT[hp][mo : mo + HD, ts(kt, P)],
                                rhs=QT[hp][mo : mo + HD, col0 : col0 + QH],
                                start=True,
                                stop=True,
                            )
                        ex = ex_pool.tile([P, HPT * QH], BF16, tag="ex")
                        nc.scalar.activation(
                            ex[:], sc[:], EXP, bias=mb[:, kt : kt + 1], scale=scale
                        )
                        for u in range(HPT):
                            h = hp * HPT + u
                            nc.tensor.matmul(
                                cxs[u][:],
                                lhsT=VP[kt][:, h * (HD + 1) : (h + 1) * (HD + 1)],
                                rhs=ex[:, ts(u, QH)],
                                start=(kt == 0),
                                stop=(kt == SKT - 1),
                            )
                        it += 1
                        if (
                            filler is not None
                            and it > pump_offset
                            and (it - pump_offset) % pump_every == 0
                        ):
                            next(filler, None)
                    prev_tails = pending[:]
                    del pending[:]
                    for u in range(HPT):
                        mo = u * HD
                        # cheap DVE copy frees the ctx PSUM slot; the rest of
                        # the normalize is deferred one unit
                        stg = small.tile(
                            [HD + 1, QH], F32, tag="stg", name="stg", bufs=4
                        )
                        nc.vector.tensor_copy(out=stg[:], in_=cxs[u][:])

                        def tail(hp=hp, mo=mo, col0=col0, stg=stg):
                            # denominator row lives on partition HD; gpsimd
                            # broadcast and custom-DVE ops only read base
                            # partition 0, so DMA it there first
                            den = small.tile([1, QH], F32, tag="den", bufs=2)
                            nc.sync.dma_start(den[0:1, :], stg[HD : HD + 1, :])
                            rec1 = small.tile([1, QH], F32, tag="rec1", bufs=2)
                            nc.vector.reciprocal_approx_fast(rec1[:], den[:])
                            rec = small.tile([HD, QH], F32, tag="rec", bufs=2)
                            nc.gpsimd.partition_broadcast(rec[:], rec1[0:1, :])
                            if mo == 0:
                                nc.vector.tensor_mul(
                                    CT[hp][0:HD, col0 : col0 + QH], stg[0:HD, :], rec[:]
                                )
                            else:
                                tmp = small.tile([HD, QH], BF16, tag="tmp")
                                nc.vector.tensor_mul(tmp[:], stg[0:HD, :], rec[:])
                                nc.sync.dma_start(
                                    CT[hp][mo : mo + HD, col0 : col0 + QH], tmp[:]
                                )

                        if defer_tails:
                            pending.append(tail)
                        else:
                            tail()
                    for fn in prev_tails:
                        fn()
                # drain remaining filler units
                if filler is not None:
                    for _ in filler:
                        pass

            def outproj_units(qq, q5, acc_pool):
                """Out-projection for the q-token window covered by (qq, q5)."""
                t0 = (qq * QS + q5 * QH) // P
                for t in range(t0, t0 + QH // P):
                    for c in range(D // OC):
                        po = acc_pool.tile([P, OC], F32, tag="acc", name="po")
                        for dd in range(MT):
                            nc.tensor.matmul(
                                po[:],
                                lhsT=CT[dd][:, ts(t, P)],
                                rhs=wos[:, dd, ts(c, OC)],
                                start=(dd == 0),
                                stop=(dd == MT - 1),
                            )
                        osb = osb_pool.tile([P, OC], F32, tag="osb")
                        nc.vector.tensor_add(osb[:], po[:], bob[:, ts(c, OC)])
                        nc.sync.dma_start(out[ts(t, P), ts(c, OC)], osb[:])
                        yield

            # ---- phase 1: V', K.T m=0, Q.T sc0 m=0 (attention prerequisites)
            with tc.tile_pool(name="ps1", bufs=2, space="PSUM") as ps1:
                for c in range(D // OC):
                    bp = ps1.tile([P, OC], F32, tag="acc", name="bp")
                    nc.tensor.matmul(
                        bp[:], lhsT=ones[0:1, 0:P], rhs=bos[0:1, ts(c, OC)],
                        start=True, stop=True,
                    )
                    nc.vector.tensor_copy(out=bob[:, ts(c, OC)], in_=bp[:])
                bp = ps1.tile([P, DL], F32, tag="acc", name="bp2")
                nc.tensor.matmul(
                    bp[:], lhsT=ones[0:1, 0:P], rhs=bvs[0:1, :], start=True, stop=True
                )
                nc.vector.tensor_copy(out=bvb[:], in_=bp[:])
                for c0, cw in kchunks:
                    proj_unit(XKT, wks, bks, KT, c0, cw, 0, ps1)
                vproj(wvs, ps1)

            # ---- phase 2: attention with projection/out-proj filler ----
            with (
                tc.tile_pool(name="ps2sc", bufs=2, space="PSUM") as ps2sc,
                tc.tile_pool(name="ps2cx", bufs=2, space="PSUM") as ps2cx,
                tc.tile_pool(name="ps2q", bufs=2, space="PSUM") as ps2q,
            ):
                CPQ = QS // NCH  # q-proj chunks per superchunk
                n_its = (HL // HPT) * SKT  # attention kt-iterations per chunk
                for c0, cw in qchunks[:CPQ]:
                    proj_unit(XQT, wqs, bqs, QT, c0, cw, 0, ps2q)

                def units(specs):
                    for XT, wsb, bsb, dst, c0, cw, m in specs:
                        proj_unit(XT, wsb, bsb, dst, c0, cw, m, ps2q)
                        yield

                def _mix(a, b):
                    from itertools import chain, zip_longest

                    yield from chain.from_iterable(zip_longest(a, b))

                # chunk sequence: (0,0), (0,1), ..., (NQ-1, NH-1).
                # filler plans keep QT[m]/KT[m] producers strictly ahead of the
                # attention head pair that consumes them (in-order PE queue).
                pending = []
                chunks2 = [(qq, q5) for qq in range(NQ) for q5 in range(NH)]
                for ci, (qq, q5) in enumerate(chunks2):
                    if ci == 0:
                        # remaining K.T slices + this chunk's Q.T column slices
                        specs = []
                        for m in range(1, MT):
                            specs.append((XQT, wqs, bqs, QT, qchunks[0][0], qchunks[0][1], m))
                            for c0, cw in kchunks:
                                specs.append((XKT, wks, bks, KT, c0, cw, m))
                        filler = units(specs)
                        pe, po = 2, 0
                    else:
                        specs = []
                        if ci == 1 and NH * NCH >= QS:
                            # q5=1 column slices for m>=1 (needed by this
                            # chunk's later head pairs)
                            for m in range(1, MT):
                                specs.append(
                                    (XQT, wqs, bqs, QT, qchunks[1][0], qchunks[1][1], m)
                                )
                        if qq + 1 < NQ and q5 == NH - 1:
                            # next superchunk's first-needed Q.T columns, all m
                            for m in range(MT):
                                specs.append(
                                    (XQT, wqs, bqs, QT, *qchunks[(qq + 1) * CPQ], m)
                                )
                        if q5 == 0 and qq >= 1:
                            # this superchunk's later Q.T columns were deferred
                            # to here; also pre-project the next superchunk's
                            # later columns if any
                            for m in range(MT):
                                for ck in qchunks[qq * CPQ + 1 : (qq + 1) * CPQ]:
                                    specs.append((XQT, wqs, bqs, QT, *ck, m))
                        pq, p5 = chunks2[ci - 1]
                        ogen = outproj_units(pq, p5, ps2q)
                        filler = _mix(units(specs), ogen) if specs else ogen
                        nun = len(specs) + QH // P * (D // OC)
                        pe, po = max(1, (n_its - 2) // nun), 2
                    attn_chunk(
                        qq, q5, ps2sc, ps2cx, pending, filler,
                        pump_every=pe, pump_offset=po,
                        defer_tails=(ci < len(chunks2) - 1),
                    )
                for fn in pending:
                    fn()
                for _ in outproj_units(NQ - 1, NH - 1, ps2q):
                    pass

    nc.compile()
    return nc


_NC_CACHE = {}


def _get_nc(S, D, DL, HD, SKT):
    key = (S, D, DL, HD, SKT)
    if key not in _NC_CACHE:
        _NC_CACHE[key] = build_nc(S, D, DL, HD, SKT)
    return _NC_CACHE[key]


def _shard_inputs(q, k, v, mask, Wq, bq, Wk, bk, Wv, bv, Wo, bo):
    q, k, v = np.asarray(q), np.asarray(k), np.asarray(v)
    mask = np.asarray(mask)
    Wq, Wk, Wv, Wo = np.asarray(Wq), np.asarray(Wk), np.asarray(Wv), np.asarray(Wo)
    bq, bk, bv, bo = np.asarray(bq), np.asarray(bk), np.asarray(bv), np.asarray(bo)

    B, S, D = q.shape  # 4, 2048, 1024
    G = 2  # head-groups (tensor-parallel factor); B*G = 8 cores
    DL = D // G
    MT = DL // P

    bf16 = ml_dtypes.bfloat16
    f32 = np.float32

    # compact keys/values: masked keys contribute exp(score-1e9) == 0 exactly
    m2 = mask[:, 0, 0, :]  # [B, S], 1 = masked
    idxs = [np.nonzero(m2[b] == 0)[0] for b in range(B)]
    SKT = max(1, -(-max(len(ix) for ix in idxs) // P))
    SK = SKT * P

    qt = [np.ascontiguousarray(q[b].T.astype(bf16)) for b in range(B)]
    kt, vt, mk = [], [], []
    for b in range(B):
        ix = idxs[b]
        n = len(ix)
        kc = np.zeros((D, SK), dtype=bf16)
        kc[:, :n] = k[b][ix].T.astype(bf16)
        vc = np.zeros((D, SK), dtype=bf16)
        vc[:, :n] = v[b][ix].T.astype(bf16)
        kt.append(kc)
        vt.append(vc)
        mk.append(
            np.ascontiguousarray(
                (np.arange(SK) >= n).astype(np.int32).reshape(SKT, P).T
            )
        )

    in_maps = []
    for c in range(B * G):
        b, g = c // G, c % G
        sl = slice(g * DL, (g + 1) * DL)
        bo_core = bo if g == 0 else np.zeros_like(bo)
        in_maps.append(
            {
                "xqt": qt[b],
                "xkt": kt[b],
                "xvt": vt[b],
                "msk": mk[b],
                "wq": np.ascontiguousarray(Wq[:, sl]).astype(bf16),
                "wk": np.ascontiguousarray(Wk[:, sl]).astype(bf16),
                "wv": np.ascontiguousarray(Wv[:, sl]).astype(bf16),
                "wo": np.ascontiguousarray(Wo[sl, :]).astype(bf16),
                "bq": np.ascontiguousarray(bq[sl].reshape(MT, P).T, dtype=f32),
                "bk": np.ascontiguousarray(bk[sl].reshape(MT, P).T, dtype=f32),
                "bv": np.ascontiguousarray(bv[sl].reshape(1, DL), dtype=f32),
                "bo": np.ascontiguousarray(bo_core.reshape(1, D), dtype=f32),
            }
        )
    return in_maps, SKT


def kernel(q, k, v, mask, Wq, bq, Wk, bk, Wv, bv, Wo, bo):
    from concourse.bass_utils import run_bass_kernel_spmd

    q = np.asarray(q)
    B, S, D = q.shape  # 4, 2048, 1024
    G = 2
    in_maps, SKT = _shard_inputs(q, k, v, mask, Wq, bq, Wk, bk, Wv, bv, Wo, bo)
    nc = _get_nc(S, D, D // G, 64, SKT)

    res = run_bass_kernel_spmd(nc, in_maps, core_ids=list(range(B * G)))
    parts = [r["out"] for r in res.results]
    outf = np.stack([parts[b * G] + parts[b * G + 1] for b in range(B)], axis=0)
    return outf.astype(np.float32)


# revision 21
# speedup vs baseline: 1.0771x; 1.0079x over previous
"""Multi-head attention (B=4, S=2048, D=1024, H=16) on 8 trn2 NeuronCores.

Sharding: 8 cores = 4 batches x 2 head-groups. Core c handles batch c//2 and
heads [8g, 8g+8) where g = c%2 (tensor-parallel: Wq/Wk/Wv column-sliced,
Wo row-sliced). Each core returns a partial output [S, D]; the host sums the
two head-group partials per batch.

Host-side prep: keys/values are COMPACTED per batch (mask==1 keys contribute
exp(score-1e9) == 0 exactly, so they are dropped and the k/v streams padded to
SK = ceil(alive/128)*128 with masked pad rows). q/k/v are transposed to
[D, S]-major on the host and everything is cast to bf16, so the device does
plain sprayed DMA loads and runs all matmuls at full bf16 PE rate.

Per-core dataflow (everything stays transposed until the output projection):
  X.T loaded directly -> Q.T/K.T = W.T @ X.T (bf16), V natural (bf16, ones
  column appended) -> scores.T = K @ Q.T (row-tiled concurrent head pairs) ->
  exp+mask+scale in one ACT op -> ctxU.T = V'.T @ expS.T (last row = softmax
  denominator) -> normalize (DMA denom to partition 0, DVE
  reciprocal_approx_fast, gpsimd broadcast) -> out = ctx.T.T @ Wo + bo.

Schedule: V and m=0 slices of K.T/Q.T are projected up front; the remaining
projection slices, then next-superchunk Q.T, then out-projection chunks are
pumped into the ACT-bound attention loop as PE filler. Attention iterates
q-chunks outermost so out-projection lags attention by one q-chunk.
"""

import sys

if "/opt/trn_rl_repo" not in sys.path:
    sys.path.append("/opt/trn_rl_repo")

import numpy as np
import ml_dtypes

import concourse.bass as bass
import concourse.bacc as bacc
import concourse.tile as tile
from concourse import mybir
from concourse.bass import ts

F32 = mybir.dt.float32
F32R = mybir.dt.float32r
BF16 = mybir.dt.bfloat16
I32 = mybir.dt.int32
EXP = mybir.ActivationFunctionType.Exp

P = 128


def build_nc(S=2048, D=1024, DL=512, HD=64, SKT=9):
    """Per-core Bass program. DL = local output dim; SKT = key token tiles."""
    SK = SKT * P  # compacted+padded key tokens
    KD = D // P  # contraction tiles over D
    MT = DL // P  # local d-col tiles
    HL = DL // HD  # local heads
    HPT = P // HD  # heads per 128-partition tile (2)
    NCH = min(512, S)  # q-projection token chunk
    QS = min(1024, S)  # attention q superchunk
    QH = min(512, QS)  # one-bank column chunk
    NH = QS // QH
    NQ = S // QS
    OC = min(512, D)  # out-proj col chunk
    kchunks = []
    off = 0
    while off < SK:
        w = min(512, SK - off)
        kchunks.append((off, w))
        off += w
    qchunks = [(i * NCH, NCH) for i in range(S // NCH)]
    scale = float(1.0 / (np.sqrt(np.float32(HD)) + 1e-8))

    nc = bacc.Bacc("TRN2", target_bir_lowering=False, debug=False)

    xqt = nc.dram_tensor("xqt", [D, S], BF16, kind="ExternalInput")
    xkt = nc.dram_tensor("xkt", [D, SK], BF16, kind="ExternalInput")
    xvt = nc.dram_tensor("xvt", [D, SK], BF16, kind="ExternalInput")
    msk = nc.dram_tensor("msk", [P, SKT], I32, kind="ExternalInput")
    wq = nc.dram_tensor("wq", [D, DL], BF16, kind="ExternalInput")
    wk = nc.dram_tensor("wk", [D, DL], BF16, kind="ExternalInput")
    wv = nc.dram_tensor("wv", [D, DL], BF16, kind="ExternalInput")
    wo = nc.dram_tensor("wo", [DL, D], BF16, kind="ExternalInput")
    bq = nc.dram_tensor("bq", [P, MT], F32, kind="ExternalInput")
    bk = nc.dram_tensor("bk", [P, MT], F32, kind="ExternalInput")
    bv = nc.dram_tensor("bv", [1, DL], F32, kind="ExternalInput")
    bo = nc.dram_tensor("bo", [1, D], F32, kind="ExternalInput")
    out = nc.dram_tensor("out", [S, D], F32, kind="ExternalOutput")

    with tile.TileContext(nc) as tc, nc.allow_low_precision("bf16 compute by design"):
        with (
            tc.tile_pool(name="pers", bufs=1) as pers,
            tc.tile_pool(name="wpool", bufs=1) as wpool,
            tc.tile_pool(name="exp", bufs=4) as ex_pool,
            tc.tile_pool(name="osb", bufs=2) as osb_pool,
            tc.tile_pool(name="small", bufs=2) as small,
        ):
            # ---- constants ----
            ones0 = pers.tile([1, P], F32, tag="ones0")
            nc.gpsimd.memset(ones0[:], 1.0)
            ones = pers.tile([1, P], F32R, tag="ones")
            nc.vector.tensor_copy(out=ones[:], in_=ones0[:])

            mi = pers.tile([P, SKT], I32, tag="mi")
            nc.sync.dma_start(mi[:], msk[:, :])
            mf = pers.tile([P, SKT], F32, tag="mf")
            nc.vector.tensor_copy(out=mf[:], in_=mi[:])
            mb = pers.tile([P, SKT], F32, tag="mb")
            nc.vector.tensor_scalar_mul(mb[:], mf[:], -1.0e9)
            wrm = pers.tile([1, P], F32, tag="wrm")
            nc.scalar.activation(wrm[:], ones0[:], EXP)

            bqs = pers.tile([P, MT], F32, tag="bqs")
            nc.sync.dma_start(bqs[:], bq[:, :])
            bks = pers.tile([P, MT], F32, tag="bks")
            nc.sync.dma_start(bks[:], bk[:, :])
            bvstg = small.tile([1, DL], F32, tag="bvstg", name="bvstg", bufs=1)
            nc.sync.dma_start(bvstg[0:1, :], bv[:, :])
            bvs = pers.tile([1, DL], F32R, tag="bvs")
            nc.vector.tensor_copy(out=bvs[:], in_=bvstg[0:1, :])
            bostg = small.tile([1, D], F32, tag="bostg", name="bostg", bufs=1)
            nc.sync.dma_start(bostg[:], bo[:, :])
            bos = pers.tile([1, D], F32R, tag="bos")
            nc.vector.tensor_copy(out=bos[:], in_=bostg[:])
            bvb = pers.tile([P, DL], F32, tag="bvb")
            bob = pers.tile([P, D], F32, tag="bob")

            # ---- X.T loads (host pre-transposed, bf16), ordered so the
            # phase-1 gates (XKT+wk, then wv+XVT, then XQT halves) land first;
            # wos (out-proj weights) is only needed mid-attention
            wks = wpool.tile([P, KD, DL], BF16, tag="w", name="w")
            wvs = wpool.tile([P, KD, DL], BF16, tag="w2", name="w2")
            wqs = wpool.tile([P, KD, DL], BF16, tag="w3", name="w3")
            XKT = pers.tile([P, KD, SK], BF16, tag="xkt")
            nc.sync.dma_start(
                wks[:, :, 0:P], wk[:, 0:P].rearrange("(k p) n -> p k n", p=P)
            )
            nc.sync.dma_start(
                XKT[:, :, 0:512], xkt[:, 0:512].rearrange("(k p) s -> p k s", p=P)
            )
            nc.sync.dma_start(
                wks[:, :, P:DL], wk[:, P:DL].rearrange("(k p) n -> p k n", p=P)
            )
            nc.sync.dma_start(
                XKT[:, :, 512:SK], xkt[:, 512:SK].rearrange("(k p) s -> p k s", p=P)
            )
            XVT = pers.tile([P, KD, SK], BF16, tag="xvt")
            nc.sync.dma_start(wvs[:], wv.rearrange("(k p) n -> p k n", p=P))
            nc.sync.dma_start(
                XVT[:, :, 0:512], xvt[:, 0:512].rearrange("(k p) s -> p k s", p=P)
            )
            nc.sync.dma_start(
                XVT[:, :, 512:SK], xvt[:, 512:SK].rearrange("(k p) s -> p k s", p=P)
            )
            XQT = pers.tile([P, KD, S], BF16, tag="xqt")
            nc.sync.dma_start(XQT[:, :, 0 : S // 2], xqt[:, 0 : S // 2].rearrange("(k p) s -> p k s", p=P))
            nc.sync.dma_start(wqs[:], wq.rearrange("(k p) n -> p k n", p=P))
            nc.sync.dma_start(XQT[:, :, S // 2 : S], xqt[:, S // 2 : S].rearrange("(k p) s -> p k s", p=P))
            wos = pers.tile([P, MT, D], BF16, tag="wos")
            nc.sync.dma_start(wos[:], wo.rearrange("(m p) n -> p m n", p=P))

            # persistent activation stores
            KT = [pers.tile([P, SK], BF16, tag=f"kt{m}", name=f"kt{m}") for m in range(MT)]
            QT = [pers.tile([P, S], BF16, tag=f"qt{m}", name=f"qt{m}") for m in range(MT)]
            CT = [pers.tile([P, S], BF16, tag=f"ct{m}", name=f"ct{m}") for m in range(MT)]
            VP = [pers.tile([P, HL * (HD + 1)], BF16, tag=f"vp{t}", name=f"vp{t}") for t in range(SKT)]
            for t in range(SKT):
                nc.gpsimd.memset(VP[t][:], 1.0)

            def proj_unit(XT, wsb, bias_sb, dst_tiles, c0, cw, m, acc_pool):
                """dst[m][:, c0:c0+cw] = ((x @ w).T + b)[m-rows, chunk]."""
                acc = acc_pool.tile([P, QH], F32, tag="acc", name="pacc")
                for kk in range(KD):
                    nc.tensor.matmul(
                        acc[:, 0:cw],
                        lhsT=wsb[:, kk, ts(m, P)],
                        rhs=XT[:, kk, c0 : c0 + cw],
                        start=(kk == 0),
                        stop=(kk == KD - 1),
                    )
                nc.vector.tensor_scalar_add(
                    dst_tiles[m][:, c0 : c0 + cw], acc[:, 0:cw], bias_sb[:, m : m + 1]
                )

            def vproj(wsb, acc_pool):
                """VP[t][:, h*(HD+1)+1:+HD] = (xv @ wv + bv)[t-tile, h-slice]."""
                for t in range(SKT):
                    acc = acc_pool.tile([P, DL], F32, tag="acc", name="vacc")
                    for kk in range(KD):
                        nc.tensor.matmul(
                            acc[:],
                            lhsT=XVT[:, kk, ts(t, P)],
                            rhs=wsb[:, kk, :],
                            start=(kk == 0),
                            stop=(kk == KD - 1),
                        )
                    for h in range(HL):
                        nc.vector.tensor_add(
                            VP[t][:, h * (HD + 1) : h * (HD + 1) + HD],
                            acc[:, ts(h, HD)],
                            bvb[:, ts(h, HD)],
                        )

            def attn_chunk(qq, q5, sc_pool, cx_pool, pending, filler=None,
                           pump_every=8, pump_offset=0, defer_tails=True):
                """Attention for q-columns [qq*QS + q5*QH, +QH), all head pairs."""
                # flush tails carried over from the previous chunk so filler
                # out-projections see completed CT columns
                for fn in pending:
                    fn()
                del pending[:]
                it = 0
                col0 = qq * QS + q5 * QH
                for hp in range(HL // HPT):
                    cxs = [
                        cx_pool.tile([HD + 1, QH], F32, tag="cx", name="cx")
                        for _ in range(HPT)
                    ]
                    for kt in range(SKT):
                        sc = sc_pool.tile([P, HPT * QH], F32, tag="sc")
                        for u in range(HPT):
                            mo = u * HD
                            nc.tensor.matmul(
                                sc[:, ts(u, QH)],
                                lhsT=KT[hp][mo : mo + HD, ts(kt, P)],
                                rhs=QT[hp][mo : mo + HD, col0 : col0 + QH],
                                start=True,
                                stop=True,
                            )
                        ex = ex_pool.tile([P, HPT * QH], BF16, tag="ex")
                        nc.scalar.activation(
                            ex[:], sc[:], EXP, bias=mb[:, kt : kt + 1], scale=scale
                        )
                        for u in range(HPT):
                            h = hp * HPT + u
                            nc.tensor.matmul(
                                cxs[u][:],
                                lhsT=VP[kt][:, h * (HD + 1) : (h + 1) * (HD + 1)],
                                rhs=ex[:, ts(u, QH)],
                                start=(kt == 0),
                                stop=(kt == SKT - 1),
                            )
                        it += 1
                        if (
                            filler is not None
                            and it > pump_offset
                            and (it - pump_offset) % pump_every == 0
                        ):
                            next(filler, None)
                    prev_tails = pending[:]
                    del pending[:]
                    for u in range(HPT):
                        mo = u * HD
                        # cheap DVE copy frees the ctx PSUM slot; the rest of
                        # the normalize is deferred one unit
                        stg = small.tile(
                            [HD + 1, QH], F32, tag="stg", name="stg", bufs=4
                        )
                        nc.vector.tensor_copy(out=stg[:], in_=cxs[u][:])

                        def tail(hp=hp, mo=mo, col0=col0, stg=stg):
                            # denominator row lives on partition HD; gpsimd
                            # broadcast and custom-DVE ops only read base
                            # partition 0, so DMA it there first
                            den = small.tile([1, QH], F32, tag="den", bufs=2)
                            nc.sync.dma_start(den[0:1, :], stg[HD : HD + 1, :])
                            rec1 = small.tile([1, QH], F32, tag="rec1", bufs=2)
                            nc.vector.reciprocal_approx_fast(rec1[:], den[:])
                            rec = small.tile([HD, QH], F32, tag="rec", bufs=2)
                            nc.gpsimd.partition_broadcast(rec[:], rec1[0:1, :])
                            if mo == 0:
                                nc.vector.tensor_mul(
                                    CT[hp][0:HD, col0 : col0 + QH], stg[0:HD, :], rec[:]
                                )
                            else:
                                tmp = small.tile([HD, QH], BF16, tag="tmp")
                                nc.vector.tensor_mul(tmp[:], stg[0:HD, :], rec[:])
                                nc.sync.dma_start(
                                    CT[hp][mo : mo + HD, col0 : col0 + QH], tmp[:]
                                )

                        if defer_tails:
                            pending.append(tail)
                        else:
                            tail()
                    for fn in prev_tails:
                        fn()
                # drain remaining filler units
                if filler is not None:
                    for _ in filler:
                        pass

            def outproj_units(qq, q5, acc_pool):
                """Out-projection for the q-token window covered by (qq, q5)."""
                t0 = (qq * QS + q5 * QH) // P
                for t in range(t0, t0 + QH // P):
                    for c in range(D // OC):
                        po = acc_pool.tile([P, OC], F32, tag="acc", name="po")
                        for dd in range(MT):
                            nc.tensor.matmul(
                                po[:],
                                lhsT=CT[dd][:, ts(t, P)],
                                rhs=wos[:, dd, ts(c, OC)],
                                start=(dd == 0),
                                stop=(dd == MT - 1),
                            )
                        osb = osb_pool.tile([P, OC], F32, tag="osb")
                        nc.vector.tensor_add(osb[:], po[:], bob[:, ts(c, OC)])
                        nc.sync.dma_start(out[ts(t, P), ts(c, OC)], osb[:])
                        yield

            # ---- phase 1: V', K.T m=0, Q.T sc0 m=0 (attention prerequisites)
            with tc.tile_pool(name="ps1", bufs=4, space="PSUM") as ps1:
                for c in range(D // OC):
                    bp = ps1.tile([P, OC], F32, tag="acc", name="bp")
                    nc.tensor.matmul(
                        bp[:], lhsT=ones[0:1, 0:P], rhs=bos[0:1, ts(c, OC)],
                        start=True, stop=True,
                    )
                    nc.vector.tensor_copy(out=bob[:, ts(c, OC)], in_=bp[:])
                bp = ps1.tile([P, DL], F32, tag="acc", name="bp2")
                nc.tensor.matmul(
                    bp[:], lhsT=ones[0:1, 0:P], rhs=bvs[0:1, :], start=True, stop=True
                )
                nc.vector.tensor_copy(out=bvb[:], in_=bp[:])
                for c0, cw in kchunks:
                    proj_unit(XKT, wks, bks, KT, c0, cw, 0, ps1)
                vproj(wvs, ps1)

            # ---- phase 2: attention with projection/out-proj filler ----
            with (
                tc.tile_pool(name="ps2sc", bufs=2, space="PSUM") as ps2sc,
                tc.tile_pool(name="ps2cx", bufs=2, space="PSUM") as ps2cx,
                tc.tile_pool(name="ps2q", bufs=2, space="PSUM") as ps2q,
            ):
                CPQ = QS // NCH  # q-proj chunks per superchunk
                n_its = (HL // HPT) * SKT  # attention kt-iterations per chunk
                for c0, cw in qchunks[:CPQ]:
                    proj_unit(XQT, wqs, bqs, QT, c0, cw, 0, ps2q)

                def units(specs):
                    for XT, wsb, bsb, dst, c0, cw, m in specs:
                        proj_unit(XT, wsb, bsb, dst, c0, cw, m, ps2q)
                        yield

                def _mix(a, b):
                    from itertools import chain, zip_longest

                    yield from chain.from_iterable(zip_longest(a, b))

                # chunk sequence: (0,0), (0,1), ..., (NQ-1, NH-1).
                # filler plans keep QT[m]/KT[m] producers strictly ahead of the
                # attention head pair that consumes them (in-order PE queue).
                pending = []
                chunks2 = [(qq, q5) for qq in range(NQ) for q5 in range(NH)]
                for ci, (qq, q5) in enumerate(chunks2):
                    if ci == 0:
                        # remaining K.T slices + this chunk's Q.T column slices
                        specs = []
                        for m in range(1, MT):
                            specs.append((XQT, wqs, bqs, QT, qchunks[0][0], qchunks[0][1], m))
                            for c0, cw in kchunks:
                                specs.append((XKT, wks, bks, KT, c0, cw, m))
                        filler = units(specs)
                        pe, po = 2, 0
                    else:
                        specs = []
                        if ci == 1 and NH * NCH >= QS:
                            # q5=1 column slices for m>=1 (needed by this
                            # chunk's later head pairs)
                            for m in range(1, MT):
                                specs.append(
                                    (XQT, wqs, bqs, QT, qchunks[1][0], qchunks[1][1], m)
                                )
                        if qq + 1 < NQ and q5 == NH - 1:
                            # next superchunk's first-needed Q.T columns, all m
                            for m in range(MT):
                                specs.append(
                                    (XQT, wqs, bqs, QT, *qchunks[(qq + 1) * CPQ], m)
                                )
                        if q5 == 0 and qq >= 1:
                            # this superchunk's later Q.T columns were deferred
                            # to here; also pre-project the next superchunk's
                            # later columns if any
                            for m in range(MT):
                                for ck in qchunks[qq * CPQ + 1 : (qq + 1) * CPQ]:
                                    specs.append((XQT, wqs, bqs, QT, *ck, m))
                        pq, p5 = chunks2[ci - 1]
                        ogen = outproj_units(pq, p5, ps2q)
                        filler = _mix(units(specs), ogen) if specs else ogen
                        nun = len(specs) + QH // P * (D // OC)
                        pe, po = max(1, (n_its - 2) // nun), 2
                    attn_chunk(
                        qq, q5, ps2sc, ps2cx, pending, filler,
                        pump_every=pe, pump_offset=po,
                        defer_tails=(ci < len(chunks2) - 1),
                    )
                for fn in pending:
                    fn()
                for _ in outproj_units(NQ - 1, NH - 1, ps2q):
                    pass

    nc.compile()
    return nc


_NC_CACHE = {}


def _get_nc(S, D, DL, HD, SKT):
    key = (S, D, DL, HD, SKT)
    if key not in _NC_CACHE:
        _NC_CACHE[key] = build_nc(S, D, DL, HD, SKT)
    return _NC_CACHE[key]


def _shard_inputs(q, k, v, mask, Wq, bq, Wk, bk, Wv, bv, Wo, bo):
    q, k, v = np.asarray(q), np.asarray(k), np.asarray(v)
    mask = np.asarray(mask)
    Wq, Wk, Wv, Wo = np.asarray(Wq), np.asarray(Wk), np.asarray(Wv), np.asarray(Wo)
    bq, bk, bv, bo = np.asarray(bq), np.asarray(bk), np.asarray(bv), np.asarray(bo)

    B, S, D = q.shape  # 4, 2048, 1024
    G = 2  # head-groups (tensor-parallel factor); B*G = 8 cores
    DL = D // G
    MT = DL // P

    bf16 = ml_dtypes.bfloat16
    f32 = np.float32

    # compact keys/values: masked keys contribute exp(score-1e9) == 0 exactly
    m2 = mask[:, 0, 0, :]  # [B, S], 1 = masked
    idxs = [np.nonzero(m2[b] == 0)[0] for b in range(B)]
    SKT = max(1, -(-max(len(ix) for ix in idxs) // P))
    SK = SKT * P

    qt = [np.ascontiguousarray(q[b].T.astype(bf16)) for b in range(B)]
    kt, vt, mk = [], [], []
    for b in range(B):
        ix = idxs[b]
        n = len(ix)
        kc = np.zeros((D, SK), dtype=bf16)
        kc[:, :n] = k[b][ix].T.astype(bf16)
        vc = np.zeros((D, SK), dtype=bf16)
        vc[:, :n] = v[b][ix].T.astype(bf16)
        kt.append(kc)
        vt.append(vc)
        mk.append(
            np.ascontiguousarray(
                (np.arange(SK) >= n).astype(np.int32).reshape(SKT, P).T
            )
        )

    in_maps = []
    for c in range(B * G):
        b, g = c // G, c % G
        sl = slice(g * DL, (g + 1) * DL)
        bo_core = bo if g == 0 else np.zeros_like(bo)
        in_maps.append(
            {
                "xqt": qt[b],
                "xkt": kt[b],
                "xvt": vt[b],
                "msk": mk[b],
                "wq": np.ascontiguousarray(Wq[:, sl]).astype(bf16),
                "wk": np.ascontiguousarray(Wk[:, sl]).astype(bf16),
                "wv": np.ascontiguousarray(Wv[:, sl]).astype(bf16),
                "wo": np.ascontiguousarray(Wo[sl, :]).astype(bf16),
                "bq": np.ascontiguousarray(bq[sl].reshape(MT, P).T, dtype=f32),
                "bk": np.ascontiguousarray(bk[sl].reshape(MT, P).T, dtype=f32),
                "bv": np.ascontiguousarray(bv[sl].reshape(1, DL), dtype=f32),
                "bo": np.ascontiguousarray(bo_core.reshape(1, D), dtype=f32),
            }
        )
    return in_maps, SKT


def kernel(q, k, v, mask, Wq, bq, Wk, bk, Wv, bv, Wo, bo):
    from concourse.bass_utils import run_bass_kernel_spmd

    q = np.asarray(q)
    B, S, D = q.shape  # 4, 2048, 1024
    G = 2
    in_maps, SKT = _shard_inputs(q, k, v, mask, Wq, bq, Wk, bk, Wv, bv, Wo, bo)
    nc = _get_nc(S, D, D // G, 64, SKT)

    res = run_bass_kernel_spmd(nc, in_maps, core_ids=list(range(B * G)))
    parts = [r["out"] for r in res.results]
    outf = np.stack([parts[b * G] + parts[b * G + 1] for b in range(B)], axis=0)
    return outf.astype(np.float32)
